# revision 1
# baseline (speedup 1.0000x reference)
"""Trainium2 Bass kernel for nn_Attention_12695923327433 (8-core SPMD).

Sharding: batch(4) x H-strips(2) -> 8 cores. Each core computes a
(384, 64, 128) slice of the output. Cross-core comm: one tiny AllReduce of
per-head gram matrices (for the l2norm + q@k attention logits) between the
two strip-cores of each batch sample.

Device algorithm per core (channels on partitions, spatial flattened on free):
u   = 1x1 conv (matmul, K=384)                      [PE]
qkv = depthwise 3x3 of u: 9 PSUM-accumulated diagonal matmuls with
        shifted rhs access patterns                    [PE]
q,k row-transposes -> DRAM spill -> gram matmuls G=[q|k][q|k]^T per head
AllReduce(G) over strip pairs; softmax blend on [48,48] tiles
attn@v and proj folded: M = proj_w @ blockdiag(attn); out = M @ v + pos
pos = dw3x3(gelu(dw3x3(v)))                          [PE + ACT]
"""
import sys
sys.path.insert(0, "/opt/trn_rl_repo")
import numpy as np
import ml_dtypes

BF = ml_dtypes.bfloat16
DIM, HEADS, NCORES = 384, 8, 8
ROWS, COLS = 70, 130          # 3+64+3 rows, 1+128+1 cols (zero-padded halo)
NPOS = ROWS * COLS            # 9100
PAD = 132                     # flat guard so shifted APs stay in-bounds
FLAT = PAD + NPOS + PAD
CH = 512

_CACHE = {}


def _build(wsm, temp, phases=7):
    import os
    SUB = int(os.environ.get("K6SUB", "9"))
    from concourse import bass, bacc, tile, mybir

    f32 = mybir.dt.float32
    bf16 = mybir.dt.bfloat16
    MM = mybir.AluOpType.mult
    ADD = mybir.AluOpType.add
    MAX = mybir.AluOpType.max
    AX = mybir.AxisListType.X
    ACT = mybir.ActivationFunctionType

    nc = bacc.Bacc("TRN2", target_bir_lowering=False, debug=False,
                   num_devices=NCORES)

    x_d = nc.dram_tensor("x", [DIM, NPOS], bf16, kind="ExternalInput")
    wT_d = nc.dram_tensor("wT", [DIM, 3 * DIM], bf16, kind="ExternalInput")
    dwd_d = nc.dram_tensor("dwd", [15, 9, 128, 128], bf16, kind="ExternalInput")
    projT_d = nc.dram_tensor("projT", [HEADS, 48, DIM], bf16, kind="ExternalInput")
    mm_d = nc.dram_tensor("maskmul", [48, HEADS * 48], f32, kind="ExternalInput")
    id_d = nc.dram_tensor("ident", [128, 128], bf16, kind="ExternalInput")
    id48_d = nc.dram_tensor("ident48", [48, 48], f32, kind="ExternalInput")
    ez_d = nc.dram_tensor("ez", [128, 2], f32, kind="ExternalInput")
    dwc_d = nc.dram_tensor("dwcol", [15, 9, 128], f32, kind="ExternalInput")
    out_d = nc.dram_tensor("out", [DIM, 64 * 128], f32, kind="ExternalOutput")
    dbg_d = nc.dram_tensor("dbg", [48, 2048], f32, kind="ExternalOutput") \
        if os.environ.get("KDBG") else None

    # flat-region chunking helpers
    full_chunks = [(s, min(NPOS, s + CH)) for s in range(0, NPOS, CH)]
    if os.environ.get("KNORESTRICT"):
        qk_u_chunks = full_chunks
        qk_dw_chunks = full_chunks
    else:
        qk_u_chunks = [(s, min(68 * COLS, s + CH))
                       for s in range(2 * COLS, 68 * COLS, CH)]
        qk_dw_chunks = [(s, min(67 * COLS, s + CH))
                        for s in range(3 * COLS, 67 * COLS, CH)]
    dw1_chunks = [(s, min(68 * COLS, s + CH)) for s in range(2 * COLS, 68 * COLS, CH)]

    with tile.TileContext(nc) as tc:
        with tc.tile_pool(name="const", bufs=1) as cp, \
             tc.tile_pool(name="persist", bufs=1) as pp, \
             tc.tile_pool(name="dramp", bufs=1, space="DRAM") as dp:

            # ---- constants ----
            wT_sb = []
            for kk in range(3):
                t = cp.tile([128, 3 * DIM], bf16, tag=f"wT{kk}", name=f"wT{kk}")
                nc.sync.dma_start(t, wT_d.ap()[128 * kk:128 * kk + 128, :])
                wT_sb.append(t)
            projT_sb = []
            for h in range(HEADS):
                t = cp.tile([48, DIM], bf16, tag=f"pjT{h}", name=f"pjT{h}")
                nc.sync.dma_start(t, projT_d.ap()[h])
                projT_sb.append(t)
            ident_sb = cp.tile([128, 128], bf16, tag="ident", name="ident")
            nc.sync.dma_start(ident_sb, id_d.ap())
            id48_sb = cp.tile([48, 48], f32, tag="id48", name="id48")
            nc.sync.dma_start(id48_sb, id48_d.ap())
            mm_sb = cp.tile([48, HEADS * 48], f32, tag="mm", name="mm")
            nc.sync.dma_start(mm_sb, mm_d.ap())
            ez_sb = cp.tile([128, 2], f32, tag="ez", name="ez")
            nc.sync.dma_start(ez_sb, ez_d.ap())

            v_sb = [pp.tile([128, FLAT], bf16, tag=f"v{i}", name=f"v{i}")
                    for i in range(3)]

            qkT_dram = dp.tile([64, 6, 128, 128], bf16, tag="qkT", name="qkT")
            n2_dram = dp.tile([6, 128], f32, tag="n2", name="n2")
            cc_in = dp.tile([48, 2048], f32, tag="ccin", name="ccin")
            cc_out = dp.tile([48, 2048], f32, tag="ccout", name="ccout")

            # ================= Phase B: conv + depthwise =================
            with tc.tile_pool(name="pb", bufs=1) as pb, \
                 tc.tile_pool(name="psB", bufs=1, space="PSUM") as psB:
                x_sb = []
                for kk in range(3):
                    t = pb.tile([128, NPOS], bf16, tag=f"x{kk}", name=f"x{kk}")
                    nc.sync.dma_start(t, x_d.ap()[128 * kk:128 * kk + 128, :])
                    x_sb.append(t)

                norm2_all = pb.tile([128, 6], f32, tag="norm2", name="norm2")
                for t9 in range(9):
                    dwd = []
                    for tap in range(9):
                        dt_ = pb.tile([128, 128], bf16, tag="dwd", bufs=9,
                                    name=f"dwd{t9}_{tap}")
                        nc.sync.dma_start(dt_, dwd_d.ap()[t9, tap])
                        dwd.append(dt_)
                    u = pb.tile([128, FLAT], bf16, tag="u", bufs=2, name=f"u{t9}")
                    if t9 < 6:
                        # rows 0..1 / 68..69 are not computed for q,k tiles;
                        # zero them (only junk dw outputs read them)
                        nc.vector.memset(u[:, 0:PAD + 2 * COLS], 0.0)
                        nc.vector.memset(u[:, PAD + 68 * COLS:FLAT], 0.0)
                    else:
                        nc.vector.memset(u[:, 0:PAD], 0.0)
                        nc.vector.memset(u[:, PAD + NPOS:FLAT], 0.0)
                    # 1x1 conv (q,k tiles only need rows 2..67)
                    for (s0, s1) in (qk_u_chunks if t9 < 6 else full_chunks):
                        n = s1 - s0
                        psA = psB.tile([128, CH], f32, tag="conv", bufs=2,
                                       name=f"psA{t9}_{s0}")
                        for kk in range(3):
                            nc.tensor.matmul(
                                psA[:, :n],
                                lhsT=wT_sb[kk][:, 128 * t9:128 * t9 + 128],
                                rhs=x_sb[kk][:, s0:s1],
                                start=(kk == 0), stop=(kk == 2))
                        nc.scalar.copy(u[:, PAD + s0:PAD + s1], psA[:, :n])
                    # depthwise 3x3 via diag matmuls
                    if t9 < 6:
                        dest = pb.tile([128, FLAT], bf16, tag="qkst", bufs=1,
                                       name=f"qkst{t9}")
                    else:
                        dest = v_sb[t9 - 6]
                    if t9 in ():
                        # q tiles: depthwise on DVE (PE is the bottleneck;
                        # bf16 accumulation noise washes out in the gram sums)
                        dwc = pb.tile([128, 9], f32, tag="dwc", bufs=2,
                                      name=f"dwc{t9}")
                        nc.sync.dma_start(dwc, dwc_d.ap()[t9].transpose([1, 0]))
                        for (s0, s1) in qk_dw_chunks:
                            n = s1 - s0
                            dst_sl = dest[:, PAD + s0:PAD + s1]
                            tmp = pb.tile([128, CH], bf16, tag="dvet", bufs=1,
                                          name=f"dvet{t9}_{s0}")
                            for tap in range(9):
                                dy, dx = tap // 3 - 1, tap % 3 - 1
                                off = PAD + s0 + dy * COLS + dx
                                if tap == 0:
                                    nc.vector.tensor_scalar_mul(
                                        dst_sl, u[:, off:off + n],
                                        dwc[:, tap:tap + 1])
                                else:
                                    nc.vector.tensor_scalar_mul(
                                        tmp[:, :n], u[:, off:off + n],
                                        dwc[:, tap:tap + 1])
                                    nc.vector.tensor_add(dst_sl, dst_sl,
                                                         tmp[:, :n])
                    else:
                      for (s0, s1) in (qk_dw_chunks if t9 < 6 else full_chunks):
                        n = s1 - s0
                        psD = psB.tile([128, CH], f32, tag="dw", bufs=2,
                                       name=f"psD{t9}_{s0}")
                        for tap in range(9):
                            dy, dx = tap // 3 - 1, tap % 3 - 1
                            off = PAD + s0 + dy * COLS + dx
                            nc.tensor.matmul(
                                psD[:, :n], lhsT=dwd[tap],
                                rhs=u[:, off:off + n],
                                start=(tap == 0), stop=(tap == 8))
                        nc.vector.tensor_copy(dest[:, PAD + s0:PAD + s1],
                                            psD[:, :n])
                    if t9 < 6 and phases >= 2:
                        # sum of squares over the valid region (l2norm diag)
                        dvv = dest[:, PAD:PAD + NPOS].rearrange(
                            "p (r c) -> p r c", c=COLS)
                        sqacc = pb.tile([128, 16], f32, tag="sqacc", bufs=2,
                                        name=f"sqa{t9}")
                        for ci in range(16):
                            sqsc = pb.tile([128, CH], bf16, tag="sqsc", bufs=1,
                                           name=f"sqs{t9}_{ci}")
                            nc.scalar.activation(
                                sqsc.rearrange("p (r c) -> p r c", r=4),
                                dvv[:, 3 + 4 * ci:7 + 4 * ci, 1:129],
                                ACT.Square, accum_out=sqacc[:, ci:ci + 1])
                        nc.vector.tensor_reduce(
                            norm2_all[:, t9:t9 + 1], sqacc, axis=AX, op=ADD)
                        # transpose valid rows, spill to DRAM (8 rows per DMA)
                        for rg in range(8):
                            stg = pb.tile([128, 8 * 128], bf16, tag="spill",
                                        bufs=2, name=f"sp{t9}_{rg}")
                            for rr in range(8):
                                r = rg * 8 + rr
                                off = PAD + (r + 3) * COLS + 1
                                psT = psB.tile([128, 128], bf16, tag="ptr",
                                               bufs=2, name=f"psT{t9}_{r}")
                                nc.tensor.transpose(psT, dest[:, off:off + 128],
                                                    ident_sb)
                                nc.vector.tensor_copy(
                                    stg[:, 128 * rr:128 * rr + 128], psT)
                            nc.sync.dma_start(
                                qkT_dram[8 * rg:8 * rg + 8, t9].transpose(
                                    [1, 0, 2]),
                                stg.rearrange("p (r c) -> p r c", r=8))
                    else:
                        vv = dest[:, PAD:PAD + NPOS].rearrange(
                            "p (r c) -> p r c", c=COLS)
                        nc.vector.memset(vv[:, :, 0:1], 0.0)
                        nc.vector.memset(vv[:, :, COLS - 1:COLS], 0.0)
                        nc.vector.tensor_scalar_mul(
                            vv[:, 2], vv[:, 2], ez_sb[:, 0:1])
                        nc.vector.tensor_scalar_mul(
                            vv[:, 67], vv[:, 67], ez_sb[:, 1:2])

                # ================= Phase C: grams =================
                G_sb = pb.tile([48, 2048], f32, tag="gsb", name="G_sb")
                if phases < 3:
                    gram_passes = []
                else:
                    gram_passes = [0]
                nc.vector.memset(G_sb, 0.0)
                for pass_ in gram_passes:
                    G_ps = psB.tile([48, 512], f32, tag="gram", bufs=1,
                                    name="G_ps")
                    for g in range(16):
                        rowg = pb.tile([128, 4 * 6 * 128], bf16, tag="rowg",
                                       bufs=2, name=f"rowg{g}")
                        nc.sync.dma_start(
                            rowg.rearrange("p (r u c) -> p r u c", r=4, u=6),
                            qkT_dram[4 * g:4 * g + 4].transpose([2, 0, 1, 3]))
                        rview = rowg.rearrange("p (r u c) -> p r u c", r=4, u=2)
                        for rr in range(4):
                            first = (g == 0 and rr == 0)
                            last = (g == 15 and rr == 3)
                            for h in range(HEADS):
                                qc = rview[:, rr, 0, 48 * h:48 * h + 48]
                                kc = rview[:, rr, 1, 48 * h:48 * h + 48]
                                # all 8 head kq-blocks share one psum bank:
                                # start/stop only on first/last touch
                                nc.tensor.matmul(
                                    G_ps[0:48, 64 * h:64 * h + 48],
                                    lhsT=kc, rhs=qc,
                                    start=(first and h == 0),
                                    stop=(last and h == 7),
                                    skip_group_check=True)
                    nc.vector.tensor_copy(
                        G_sb.rearrange("p (h c) -> p h c", h=HEADS)[:, :, 0:48],
                        G_ps.rearrange("p (h c) -> p h c", h=HEADS)[:, :, 0:48])
                    # pack l2norm sums into the payload: per-head cols
                    # [48]=qnorm2, [49]=knorm2
                    nc.sync.dma_start(n2_dram.transpose([1, 0]), norm2_all)
                    n2flat = n2_dram.rearrange("t p -> (t p)").rearrange(
                        "(u h c) -> u h c", u=2, h=HEADS)
                    Gsbv = G_sb.rearrange("p (h c) -> p h c", h=HEADS)
                    nc.sync.dma_start(Gsbv[:, :, 48:49].opt(),
                                      n2flat[0].transpose([1, 0]).opt())
                    nc.sync.dma_start(Gsbv[:, :, 49:50].opt(),
                                      n2flat[1].transpose([1, 0]).opt())
                # ================= Phase D: AllReduce =================
                if phases >= 4:
                    nc.sync.dma_start(cc_in, G_sb)
                    nc.gpsimd.collective_compute(
                        "AllReduce", ADD,
                        replica_groups=[[0, 1], [2, 3], [4, 5], [6, 7]],
                        ins=[cc_in.opt()], outs=[cc_out.opt()])

            # ============ late phases share SBUF freed by phase B ============
            with tc.tile_pool(name="late", bufs=1) as lp:
                pos_sb = [lp.tile([128, 64 * 128], bf16, tag=f"pos{i}",
                                name=f"pos{i}") for i in range(3)]
                MT_sb = [lp.tile([128, DIM], bf16, tag=f"mt{i}", name=f"mtl{i}")
                         for i in range(3)]
                G_sum = lp.tile([48, 2048], f32, tag="gsum", name="gsum")
                attn_bf = [lp.tile([48, 48], bf16, tag=f"at{h}", name=f"atl{h}")
                           for h in range(HEADS)]
                if phases >= 4:
                    nc.sync.dma_start(G_sum, cc_out)
                if dbg_d is not None:
                    nc.sync.dma_start(dbg_d.ap(), G_sum)
                # ================= Phase G: pos path =================
                with tc.tile_pool(name="pg", bufs=1) as pg, \
                     tc.tile_pool(name="psG", bufs=1, space="PSUM") as psG:
                    for vt in (range(3) if phases >= 5 else []):
                        dwd1 = []
                        for tap in range(9):
                            dt_ = pg.tile([128, 128], bf16, tag="dwdg", bufs=18,
                                            name=f"dwp1_{vt}_{tap}")
                            nc.sync.dma_start(dt_, dwd_d.ap()[9 + vt, tap])
                            dwd1.append(dt_)
                        g1 = pg.tile([128, FLAT], bf16, tag="g1", bufs=2,
                                       name=f"g1_{vt}")
                        for (s0, s1) in dw1_chunks:
                            n = s1 - s0
                            psP = psG.tile([128, CH], f32, tag="dwg", bufs=2,
                                             name=f"psP{vt}_{s0}")
                            for tap in range(9):
                                dy, dx = tap // 3 - 1, tap % 3 - 1
                                off = PAD + s0 + dy * COLS + dx
                                nc.tensor.matmul(
                                    psP[:, :n], lhsT=dwd1[tap],
                                    rhs=v_sb[vt][:, off:off + n],
                                    start=(tap == 0), stop=(tap == 8))
                            nc.scalar.activation(g1[:, PAD + s0:PAD + s1],
                                                   psP[:, :n], ACT.Gelu)
                        g1v = g1[:, PAD:PAD + NPOS].rearrange(
                            "p (r c) -> p r c", c=COLS)
                        nc.vector.memset(g1v[:, 2:68, 0:1], 0.0)
                        nc.vector.memset(g1v[:, 2:68, COLS - 1:COLS], 0.0)
                        nc.vector.tensor_scalar_mul(
                            g1v[:, 2], g1v[:, 2], ez_sb[:, 0:1])
                        nc.vector.tensor_scalar_mul(
                            g1v[:, 67], g1v[:, 67], ez_sb[:, 1:2])
                        dwd2 = []
                        for tap in (range(9) if vt != 2 else []):
                            dt_ = pg.tile([128, 128], bf16, tag="dwdg", bufs=18,
                                            name=f"dwp2_{vt}_{tap}")
                            nc.sync.dma_start(dt_, dwd_d.ap()[12 + vt, tap])
                            dwd2.append(dt_)
                        if vt == 2:
                            dwc2 = pg.tile([128, 9], f32, tag="dwc2", bufs=1,
                                           name=f"dwc2v{vt}")
                            nc.sync.dma_start(
                                dwc2, dwc_d.ap()[12 + vt].transpose([1, 0]))
                        for i in range(16):
                            if vt == 2:
                                dst_sl = pos_sb[vt][:, CH * i:CH * i + CH]
                                dview = dst_sl.rearrange("p (r c) -> p r c", r=4)
                                tmp2 = pg.tile([128, CH], bf16, tag="dvet2",
                                               bufs=2, name=f"dvet2_{i}")
                                tview = tmp2.rearrange("p (r c) -> p r c", r=4)
                                for tap in range(9):
                                    dy, dx = tap // 3 - 1, tap % 3 - 1
                                    rhs = g1v[:, 3 + 4 * i + dy:7 + 4 * i + dy,
                                              1 + dx:129 + dx]
                                    if tap == 0:
                                        nc.vector.tensor_scalar_mul(
                                            dview, rhs, dwc2[:, tap:tap + 1])
                                    else:
                                        nc.vector.tensor_scalar_mul(
                                            tview, rhs, dwc2[:, tap:tap + 1])
                                        nc.vector.tensor_add(dst_sl, dst_sl,
                                                             tmp2)
                                continue
                            psQ = psG.tile([128, CH], f32, tag="dwg", bufs=2,
                                           name=f"psQ{vt}_{i}")
                            for tap in range(9):
                                dy, dx = tap // 3 - 1, tap % 3 - 1
                                rhs = g1v[:, 3 + 4 * i + dy:7 + 4 * i + dy,
                                          1 + dx:129 + dx]
                                nc.tensor.matmul(psQ, lhsT=dwd2[tap], rhs=rhs,
                                                 start=(tap == 0),
                                                 stop=(tap == 8))
                            nc.vector.tensor_copy(
                                pos_sb[vt][:, CH * i:CH * i + CH], psQ)

                # ================= Phase E: softmax / attn =================
                with tc.tile_pool(name="pe", bufs=1) as pe, \
                     tc.tile_pool(name="psE", bufs=1, space="PSUM") as psE:
                    # per head block: cols [0:48)=kq (d rows, c cols),
                    # [48:96)=qq, [96:144)=kk
                    Gv = G_sum.rearrange("p (h c) -> p h c", h=HEADS)
                    nrm = pe.tile([48, 16], f32, tag="nrm", name="nrm")
                    inv = pe.tile([48, 16], f32, tag="inv", name="inv")
                    if phases >= 6 and SUB >= 2:
                        # cols 2h = qnorm, 2h+1 = knorm
                        nc.scalar.sqrt(
                            nrm.rearrange("p (h u) -> p h u", h=HEADS),
                            Gv[:, :, 48:50])
                        nc.vector.tensor_scalar_max(nrm, nrm, 1e-12)
                        nc.vector.reciprocal(inv, nrm)

                    for h in (range(HEADS) if phases >= 6 and SUB >= 3 else []):
                        B = pe.tile([48, 48], f32, tag="B", bufs=2, name=f"B{h}")
                        nc.vector.tensor_scalar(
                            out=B, in0=Gv[:, h, 0:48],
                            scalar1=inv[:, 2 * h + 1:2 * h + 2],
                            scalar2=float(temp[h]),
                            op0=MM, op1=MM)
                        psb = psE.tile([48, 48], f32, tag="ptr2", bufs=2,
                                         name=f"psb{h}")
                        nc.tensor.transpose(psb, B, id48_sb)
                        A0 = pe.tile([48, 48], f32, tag="A0", bufs=2, name=f"A0{h}")
                        nc.vector.tensor_scalar_mul(
                            A0, psb, inv[:, 2 * h:2 * h + 1])
                        e0 = pe.tile([48, 48], f32, tag="e0", bufs=2, name=f"e0{h}")
                        s_ = pe.tile([48, 4], f32, tag="s", bufs=2, name=f"s{h}")
                        nc.scalar.activation(e0, A0, ACT.Exp,
                                               accum_out=s_[:, 0:1])
                        if SUB < 4:
                            continue
                        e1 = pe.tile([48, 48], f32, tag="e1", bufs=2, name=f"e1{h}")
                        nc.vector.tensor_mul(e1, e0, mm_sb[:, 48 * h:48 * h + 48])
                        nc.vector.tensor_reduce(
                            s_[:, 1:2], e1, axis=AX, op=ADD)
                        r_ = pe.tile([48, 4], f32, tag="r", bufs=2, name=f"r{h}")
                        nc.vector.reciprocal(r_[:, 0:2], s_[:, 0:2])
                        nc.vector.tensor_scalar_mul(r_[:, 0:1], r_[:, 0:1],
                                                    float(wsm[0]))
                        nc.vector.tensor_scalar_mul(r_[:, 1:2], r_[:, 1:2],
                                                    float(wsm[1]))
                        t0 = pe.tile([48, 48], f32, tag="t0", bufs=2, name=f"t0{h}")
                        nc.vector.tensor_scalar_mul(t0, e0, r_[:, 0:1])
                        af = pe.tile([48, 48], f32, tag="af", bufs=2, name=f"af{h}")
                        nc.vector.tensor_scalar_mul(af, e1, r_[:, 1:2])
                        nc.vector.tensor_add(af, af, t0)
                        nc.vector.tensor_copy(attn_bf[h], af)

                    # ---- Phase F: M = proj @ blockdiag(attn), transposed ----
                    M_ps = [psE.tile([128, DIM], f32, tag=f"mps{ot}", name=f"mps{ot}")
                            for ot in range(3)]
                    for h in (range(HEADS) if phases >= 6 and SUB >= 5 else []):
                        for ot in range(3):
                            nc.tensor.matmul(
                                M_ps[ot][:, 48 * h:48 * h + 48],
                                lhsT=projT_sb[h][:, 128 * ot:128 * ot + 128],
                                rhs=attn_bf[h], start=True, stop=True)
                    M_sb = []
                    for ot in (range(3) if phases >= 6 and SUB >= 6 else []):
                        t = pe.tile([128, DIM], bf16, tag=f"msb{ot}", name=f"msb{ot}")
                        nc.vector.tensor_copy(t, M_ps[ot])
                        M_sb.append(t)
                    for ot in (range(3) if phases >= 6 and SUB >= 7 else []):
                        for dtt in range(3):
                            psM = psE.tile([128, 128], bf16, tag="ptr3", bufs=2,
                                             name=f"psM{ot}_{dtt}")
                            nc.tensor.transpose(
                                psM, M_sb[ot][:, 128 * dtt:128 * dtt + 128],
                                ident_sb)
                            nc.vector.tensor_copy(
                                MT_sb[dtt][:, 128 * ot:128 * ot + 128], psM)

                # ================= Phase H: out = M @ v + pos =================
                with tc.tile_pool(name="ph", bufs=1) as ph, \
                     tc.tile_pool(name="psH", bufs=1, space="PSUM") as psH:
                    vviews = [v_sb[i][:, PAD:PAD + NPOS].rearrange(
                        "p (r c) -> p r c", c=COLS) for i in range(3)]
                    for ot in (range(3) if phases >= 7 else []):
                        for i in range(16):
                            psO = psH.tile([128, CH], f32, tag="out", bufs=2,
                                             name=f"psO{ot}_{i}")
                            for dtt in range(3):
                                rhs = vviews[dtt][:, 3 + 4 * i:7 + 4 * i, 1:129]
                                nc.tensor.matmul(
                                    psO, lhsT=MT_sb[dtt][:, 128 * ot:128 * ot + 128],
                                    rhs=rhs, start=(dtt == 0), stop=(dtt == 2))
                            outf = ph.tile([128, CH], f32, tag="outf", bufs=3,
                                             name=f"outf{ot}_{i}")
                            nc.vector.tensor_add(
                                outf, psO, pos_sb[ot][:, CH * i:CH * i + CH])
                            nc.sync.dma_start(
                                out_d.ap()[128 * ot:128 * ot + 128,
                                             CH * i:CH * i + CH], outf)

    nc.compile()
    return nc


def _host_prep(x, mask, qkv_w, dw_w, proj_w, temperature, w_blend,
               pos_w1, pos_w2):
    x = np.asarray(x, np.float32)
    b = x.shape[0]
    xp = np.zeros((b, DIM, 134, COLS), np.float32)
    xp[:, :, 3:131, 1:129] = x
    shards = []
    for core in range(NCORES):
        bi, s = core // 2, core % 2
        shards.append(np.ascontiguousarray(
            xp[bi, :, 64 * s:64 * s + ROWS, :]).reshape(DIM, NPOS).astype(BF))
    wT = np.ascontiguousarray(
        np.asarray(qkv_w, np.float32)[:, :, 0, 0].T).astype(BF)
    dwd = np.zeros((15, 9, 128, 128), np.float32)
    dwk = np.asarray(dw_w, np.float32)[:, 0]       # (1152, 3, 3)
    pk1 = np.asarray(pos_w1, np.float32)[:, 0]     # (384, 3, 3)
    pk2 = np.asarray(pos_w2, np.float32)[:, 0]
    idx = np.arange(128)
    for t9 in range(9):
        for tap in range(9):
            dwd[t9, tap, idx, idx] = dwk[128 * t9:128 * t9 + 128,
                                         tap // 3, tap % 3]
    for vt in range(3):
        for tap in range(9):
            dwd[9 + vt, tap, idx, idx] = pk1[128 * vt:128 * vt + 128,
                                             tap // 3, tap % 3]
            dwd[12 + vt, tap, idx, idx] = pk2[128 * vt:128 * vt + 128,
                                            tap // 3, tap % 3]
    dwcol = np.zeros((15, 9, 128), np.float32)
    for t9 in range(9):
        for tap in range(9):
            dwcol[t9, tap] = dwd[t9, tap, idx, idx]
    for vt in range(3):
        for tap in range(9):
            dwcol[9 + vt, tap] = dwd[9 + vt, tap, idx, idx]
            dwcol[12 + vt, tap] = dwd[12 + vt, tap, idx, idx]
    dwd = dwd.astype(BF)
    pw = np.asarray(proj_w, np.float32)[:, :, 0, 0]
    projT = np.stack([np.ascontiguousarray(pw[:, 48 * h:48 * h + 48].T)
                    for h in range(HEADS)]).astype(BF)     # (8, 48, 384)
    mmul = (np.asarray(mask)[0] != 0).astype(np.float32)     # (8, 48, 48)
    maskmul = np.ascontiguousarray(
        mmul.transpose(1, 0, 2).reshape(48, HEADS * 48))
    ident = np.eye(128, dtype=np.float32).astype(BF)
    ident48 = np.eye(48, dtype=np.float32)
    wb = np.asarray(w_blend, np.float32)
    e = np.exp(wb - wb.max())
    wsm = e / e.sum()
    temp = np.asarray(temperature, np.float32).reshape(HEADS)
    return (shards, wT, dwd, dwcol, projT, maskmul, ident, ident48, wsm,
            temp)


def kernel(**inputs):
    from concourse import bass_utils
    (shards, wT, dwd, dwcol, projT, maskmul, ident, ident48, wsm,
     temp) = _host_prep(**inputs)
    key = (tuple(np.round(wsm, 8)), tuple(np.round(temp, 8)))
    if key not in _CACHE:
        _CACHE[key] = _build(wsm, temp)
    nc = _CACHE[key]
    in_maps = []
    for core in range(NCORES):
        s = core % 2
        ez = np.ones((128, 2), np.float32)
        ez[:, 0] = 0.0 if s == 0 else 1.0
        ez[:, 1] = 0.0 if s == 1 else 1.0
        in_maps.append({
            "x": shards[core], "wT": wT, "dwd": dwd, "dwcol": dwcol,
            "projT": projT, "maskmul": maskmul, "ident": ident,
            "ident48": ident48, "ez": ez,
        })
    res = bass_utils.run_bass_kernel_spmd(
        nc, in_maps, core_ids=list(range(NCORES)),
        trace=bool(int(__import__("os").environ.get("KBENCH_TRACE", "0"))))
    kernel._last_result = res
    x = np.asarray(inputs["x"])
    out = np.zeros((x.shape[0], DIM, 128, 128), np.float32)
    for core in range(NCORES):
        bi, s = core // 2, core % 2
        out[bi, :, 64 * s:64 * s + 64, :] = \
            np.asarray(res.results[core]["out"], np.float32).reshape(
                DIM, 64, 128)
    return out



# revision 7
# speedup vs baseline: 1.1295x; 1.1295x over previous
"""Trainium2 Bass kernel for nn_Attention_12695923327433 (8-core SPMD).

Sharding: batch(4) x H-strips(2) -> 8 cores. Each core computes a
(384, 64, 128) slice of the output. Cross-core comm: one tiny AllReduce of
per-head gram matrices (for the l2norm + q@k attention logits) between the
two strip-cores of each batch sample.

Device algorithm per core (channels on partitions, spatial flattened on free):
u   = 1x1 conv (matmul, K=384)                      [PE]
qkv = depthwise 3x3 of u: 9 PSUM-accumulated diagonal matmuls with
        shifted rhs access patterns                    [PE]
q,k tiles spilled contiguously to DRAM; gram phase reloads them through
  the DMA xbar transpose (dma_start_transpose) and accumulates per-head
  48x48 k^T q blocks with col-tiled matmul pairs (2 heads concurrently).
AllReduce(G) over strip pairs; softmax blend on [48,48] tiles.
M = proj_w @ blockdiag(attn); out chunks are single PSUM groups fusing
  the 9 pos-dw2 diagonal taps (on gelu(dw1(v))) with the 3 M@v matmuls.
"""
import sys
sys.path.insert(0, "/opt/trn_rl_repo")
import numpy as np
import ml_dtypes

BF = ml_dtypes.bfloat16
DIM, HEADS, NCORES = 384, 8, 8
ROWS, COLS = 70, 130          # 3+64+3 rows, 1+128+1 cols (zero-padded halo)
NPOS = ROWS * COLS            # 9100
PAD = 132                     # flat guard so shifted APs stay in-bounds
FLAT = PAD + NPOS + PAD
CH = 512

_CACHE = {}


def _build(wsm, temp):
    from concourse import bass, bacc, tile, mybir

    f32 = mybir.dt.float32
    bf16 = mybir.dt.bfloat16
    MM = mybir.AluOpType.mult
    ADD = mybir.AluOpType.add
    AX = mybir.AxisListType.X
    ACT = mybir.ActivationFunctionType

    nc = bacc.Bacc("TRN2", target_bir_lowering=False, debug=False,
                   num_devices=NCORES)

    x_d = nc.dram_tensor("x", [DIM, NPOS], bf16, kind="ExternalInput")
    wT_d = nc.dram_tensor("wT", [DIM, 3 * DIM], bf16, kind="ExternalInput")
    dwd_d = nc.dram_tensor("dwd", [15, 9, 128, 128], bf16, kind="ExternalInput")
    projT_d = nc.dram_tensor("projT", [HEADS, 48, DIM], bf16, kind="ExternalInput")
    mm_d = nc.dram_tensor("maskmul", [48, HEADS * 48], f32, kind="ExternalInput")
    id_d = nc.dram_tensor("ident", [128, 128], bf16, kind="ExternalInput")
    id48_d = nc.dram_tensor("ident48", [48, 48], f32, kind="ExternalInput")
    ez_d = nc.dram_tensor("ez", [128, 2], f32, kind="ExternalInput")
    out_d = nc.dram_tensor("out", [DIM, 64 * 128], f32, kind="ExternalOutput")

    # flat-region chunking (rows of the 70x130 halo grid, 512-wide chunks)
    full_chunks = [(s, min(NPOS, s + CH)) for s in range(0, NPOS, CH)]
    qk_u_chunks = [(s, min(68 * COLS, s + CH))
                   for s in range(2 * COLS, 68 * COLS, CH)]
    qk_dw_chunks = [(s, min(67 * COLS, s + CH))
                    for s in range(3 * COLS, 67 * COLS, CH)]
    dw1_chunks = [(s, min(68 * COLS, s + CH)) for s in range(2 * COLS, 68 * COLS, CH)]

    # payload block for head h (even heads on psum partitions 0-47,
    # odd heads on 64-111; payload blocks even-first)
    def blk(h):
        return (h % 2) * 4 + h // 2

    with tile.TileContext(nc) as tc:
        with tc.tile_pool(name="const", bufs=1) as cp, \
             tc.tile_pool(name="persist", bufs=1) as pp, \
             tc.tile_pool(name="dramp", bufs=1, space="DRAM") as dp:

            # ---- constants ----
            wT_sb = []
            for kk in range(3):
                t = cp.tile([128, 3 * DIM], bf16, tag=f"wT{kk}", name=f"wT{kk}")
                nc.sync.dma_start(t, wT_d.ap()[128 * kk:128 * kk + 128, :])
                wT_sb.append(t)
            projT_sb = []
            for h in range(HEADS):
                t = cp.tile([48, DIM], bf16, tag=f"pjT{h}", name=f"pjT{h}")
                nc.sync.dma_start(t, projT_d.ap()[h])
                projT_sb.append(t)
            ident_sb = cp.tile([128, 128], bf16, tag="ident", name="ident")
            nc.sync.dma_start(ident_sb, id_d.ap())
            id48_sb = cp.tile([48, 48], f32, tag="id48", name="id48")
            nc.sync.dma_start(id48_sb, id48_d.ap())
            mm_sb = cp.tile([48, HEADS * 48], f32, tag="mm", name="mm")
            nc.sync.dma_start(mm_sb, mm_d.ap())
            ez_sb = cp.tile([128, 2], f32, tag="ez", name="ez")
            nc.sync.dma_start(ez_sb, ez_d.ap())

            v_sb = [None] * 3
            G_sb = pp.tile([48, 2048], f32, tag="gsb", name="G_sb")

            qk_dram = dp.tile([6, 128, 64 * 128], bf16, tag="qkspill",
                              name="qk_dram")
            n2_dram = dp.tile([6, 128], f32, tag="n2", name="n2")
            cc_in = dp.tile([48, 2048], f32, tag="ccin", name="ccin")
            cc_out = dp.tile([48, 2048], f32, tag="ccout", name="ccout")

            # ================= Phase B + gram =================
            with tc.tile_pool(name="pb", bufs=1) as pb, \
                 tc.tile_pool(name="psB", bufs=1, space="PSUM") as psB:
                x_sb = []
                for kk in range(3):
                    t = pb.tile([128, NPOS], bf16, tag=f"x{kk}", name=f"x{kk}")
                    # chunked loads so the first conv can start early
                    for q4 in range(4):
                        s = (NPOS // 4) * q4
                        e = NPOS if q4 == 3 else (NPOS // 4) * (q4 + 1)
                        nc.sync.dma_start(t[:, s:e], x_d.ap()[128 * kk:128 * kk + 128, s:e])
                    x_sb.append(t)

                norm2_all = pb.tile([128, 6], f32, tag="norm2", name="norm2")

                def conv_dw_tile(t9, dest, u_chunks, dw_chunks):
                    dwd = []
                    for tap in range(9):
                        dt_ = pb.tile([128, 128], bf16, tag="dwd", bufs=18,
                                      name=f"dwd{t9}_{tap}")
                        nc.sync.dma_start(dt_, dwd_d.ap()[t9, tap])
                        dwd.append(dt_)
                    u = pb.tile([128, FLAT], bf16, tag="u", bufs=1, name=f"u{t9}")
                    if t9 < 6:
                        nc.vector.memset(u[:, 0:PAD + 2 * COLS], 0.0)
                        nc.vector.memset(u[:, PAD + 68 * COLS:FLAT], 0.0)
                    else:
                        nc.vector.memset(u[:, 0:PAD], 0.0)
                        nc.vector.memset(u[:, PAD + NPOS:FLAT], 0.0)
                    for (s0, s1) in u_chunks:
                        n = s1 - s0
                        psA = psB.tile([128, CH], f32, tag="conv", bufs=2,
                                       name=f"psA{t9}_{s0}")
                        for kk in range(3):
                            nc.tensor.matmul(
                                psA[:, :n],
                                lhsT=wT_sb[kk][:, 128 * t9:128 * t9 + 128],
                                rhs=x_sb[kk][:, s0:s1],
                                start=(kk == 0), stop=(kk == 2))
                        nc.scalar.copy(u[:, PAD + s0:PAD + s1], psA[:, :n])
                    for (s0, s1) in dw_chunks:
                        n = s1 - s0
                        psD = psB.tile([128, CH], f32, tag="dw", bufs=2,
                                       name=f"psD{t9}_{s0}")
                        for tap in range(9):
                            dy, dx = tap // 3 - 1, tap % 3 - 1
                            off = PAD + s0 + dy * COLS + dx
                            nc.tensor.matmul(
                                psD[:, :n], lhsT=dwd[tap],
                                rhs=u[:, off:off + n],
                                start=(tap == 0), stop=(tap == 8))
                        nc.vector.tensor_copy(dest[:, PAD + s0:PAD + s1],
                                              psD[:, :n])

                # ---- q,k tiles first ----
                for t9 in range(6):
                    dest = pb.tile([128, FLAT], bf16, tag="qkst", bufs=2,
                                   name=f"qkst{t9}")
                    conv_dw_tile(t9, dest, qk_u_chunks, qk_dw_chunks)
                    dvv = dest[:, PAD:PAD + NPOS].rearrange(
                        "p (r c) -> p r c", c=COLS)
                    # sum of squares over the valid region (l2norm diag)
                    sqacc = pb.tile([128, 16], f32, tag="sqacc", bufs=2,
                                    name=f"sqa{t9}")
                    for ci in range(16):
                        sqsc = pb.tile([128, CH], bf16, tag="sqsc", bufs=1,
                                       name=f"sqs{t9}_{ci}")
                        nc.scalar.activation(
                            sqsc.rearrange("p (r c) -> p r c", r=4),
                            dvv[:, 3 + 4 * ci:7 + 4 * ci, 1:129],
                            ACT.Square, accum_out=sqacc[:, ci:ci + 1])
                    nc.vector.tensor_reduce(
                        norm2_all[:, t9:t9 + 1], sqacc, axis=AX, op=ADD)
                    # contiguous spill of the valid region
                    nc.sync.dma_start(qk_dram[t9], dvv[:, 3:67, 1:129])

                # ---- gram: xbar-transposed reload + col-tiled matmuls ----
                # (overlaps the v-tile conv/dw below via the scheduler)
                G_ps = psB.tile([128, 256], f32, tag="gram", bufs=1,
                                name="G_ps")
                for g in range(16):
                    stg = pb.tile([128, 4, 6, 128], bf16, tag="stage", bufs=2,
                                  name=f"stg{g}")
                    for t in range(6):
                        nc.sync.dma_start_transpose(
                            stg[:, :, t, :],
                            qk_dram[t, :, CH * g:CH * g + CH])
                    stv = stg.rearrange("p a t c -> p a (t c)")
                    for m in range(4):
                        for h in range(HEADS):
                            base = (h % 2) * 64
                            qc = stv[:, m, 48 * h:48 * h + 48]
                            kc = stv[:, m, 384 + 48 * h:384 + 48 * h + 48]
                            nc.tensor.matmul(
                                G_ps[base:base + 48,
                                     64 * (h // 2):64 * (h // 2) + 48],
                                lhsT=kc, rhs=qc,
                                start=(g == 0 and m == 0 and h < 2),
                                stop=(g == 15 and m == 3 and h >= 6),
                                tile_position=(0, base),
                                skip_group_check=True)

                # ---- v tiles (concurrent with gram on the trace timeline) ----
                for t9 in range(6, 9):
                    dest = pp.tile([128, FLAT], bf16, tag=f"v{t9-6}",
                                   name=f"v{t9-6}")
                    v_sb[t9 - 6] = dest
                    conv_dw_tile(t9, dest, full_chunks, full_chunks)
                    vv = dest[:, PAD:PAD + NPOS].rearrange(
                        "p (r c) -> p r c", c=COLS)
                    nc.vector.memset(vv[:, :, 0:1], 0.0)
                    nc.vector.memset(vv[:, :, COLS - 1:COLS], 0.0)
                    nc.vector.tensor_scalar_mul(
                        vv[:, 2], vv[:, 2], ez_sb[:, 0:1])
                    nc.vector.tensor_scalar_mul(
                        vv[:, 67], vv[:, 67], ez_sb[:, 1:2])

                # ---- payload: gram blocks + l2norm sums ----
                Gsbv = G_sb.rearrange("p (b c) -> p b c", b=HEADS)
                nc.vector.tensor_copy(
                    Gsbv[0:48, 0:4, 0:48],
                    G_ps[0:48].rearrange("p (j c) -> p j c", j=4)[:, :, 0:48])
                nc.vector.tensor_copy(
                    Gsbv[0:48, 4:8, 0:48],
                    G_ps[64:112].rearrange("p (j c) -> p j c", j=4)[:, :, 0:48])
                nc.sync.dma_start(n2_dram.transpose([1, 0]), norm2_all)
                n2flat = n2_dram.rearrange("t p -> (t p)").rearrange(
                    "(u h c) -> u h c", u=2, h=HEADS)
                # payload col 48 = qnorm2, col 49 = knorm2; head h -> block
                # (h%2)*4 + h//2, i.e. src head rows reordered by parity
                for u2 in range(2):
                    for par in range(2):
                        src = n2flat[u2].rearrange(
                            "(j p2) c -> j p2 c", p2=2)[:, par]
                        nc.sync.dma_start(
                            Gsbv[:, 4 * par:4 * par + 4, 48 + u2:49 + u2].opt(),
                            src.transpose([1, 0]).opt())

                # ================= AllReduce =================
                nc.sync.dma_start(cc_in, G_sb)
                nc.gpsimd.collective_compute(
                    "AllReduce", ADD,
                    replica_groups=[[0, 1], [2, 3], [4, 5], [6, 7]],
                    ins=[cc_in.opt()], outs=[cc_out.opt()])

            # ============ late phases share SBUF freed by phase B ============
            with tc.tile_pool(name="late", bufs=1) as lp:
                g1_sb = [lp.tile([128, FLAT], bf16, tag=f"g1_{i}",
                                 name=f"g1_{i}") for i in range(3)]
                MT_sb = [lp.tile([128, DIM], bf16, tag=f"mt{i}", name=f"mtl{i}")
                         for i in range(3)]
                G_sum = lp.tile([48, 2048], f32, tag="gsum", name="gsum")
                attn_bf = [lp.tile([48, 48], bf16, tag=f"at{h}", name=f"atl{h}")
                           for h in range(HEADS)]
                nc.sync.dma_start(G_sum, cc_out)
                # pos-path depthwise taps (dw1 for phase G, dw2 fused into out)
                dwd1_sb = [[None] * 9 for _ in range(3)]
                dwd2_sb = [[None] * 9 for _ in range(3)]
                for vt in range(3):
                    for tap in range(9):
                        t1 = lp.tile([128, 128], bf16, tag="dwp1", bufs=27,
                                     name=f"dwp1_{vt}_{tap}")
                        nc.sync.dma_start(t1, dwd_d.ap()[9 + vt, tap])
                        dwd1_sb[vt][tap] = t1
                        t2 = lp.tile([128, 128], bf16, tag="dwp2", bufs=27,
                                     name=f"dwp2_{vt}_{tap}")
                        nc.sync.dma_start(t2, dwd_d.ap()[12 + vt, tap])
                        dwd2_sb[vt][tap] = t2

                # ================= pos path dw1 + gelu =================
                with tc.tile_pool(name="pg", bufs=1) as pg, \
                     tc.tile_pool(name="psG", bufs=1, space="PSUM") as psG:
                    for vt in range(3):
                        g1 = g1_sb[vt]
                        nc.vector.memset(g1[:, 0:PAD + 2 * COLS], 0.0)
                        nc.vector.memset(g1[:, PAD + 68 * COLS:FLAT], 0.0)
                        for (s0, s1) in dw1_chunks:
                            n = s1 - s0
                            psP = psG.tile([128, CH], f32, tag="dwg", bufs=2,
                                           name=f"psP{vt}_{s0}")
                            for tap in range(9):
                                dy, dx = tap // 3 - 1, tap % 3 - 1
                                off = PAD + s0 + dy * COLS + dx
                                nc.tensor.matmul(
                                    psP[:, :n], lhsT=dwd1_sb[vt][tap],
                                    rhs=v_sb[vt][:, off:off + n],
                                    start=(tap == 0), stop=(tap == 8))
                            nc.scalar.activation(g1[:, PAD + s0:PAD + s1],
                                                 psP[:, :n], ACT.Gelu)
                        g1v = g1[:, PAD:PAD + NPOS].rearrange(
                            "p (r c) -> p r c", c=COLS)
                        nc.vector.memset(g1v[:, 2:68, 0:1], 0.0)
                        nc.vector.memset(g1v[:, 2:68, COLS - 1:COLS], 0.0)
                        nc.vector.tensor_scalar_mul(
                            g1v[:, 2], g1v[:, 2], ez_sb[:, 0:1])
                        nc.vector.tensor_scalar_mul(
                            g1v[:, 67], g1v[:, 67], ez_sb[:, 1:2])

                    # ============ softmax / attn / M build ============
                    with tc.tile_pool(name="pe", bufs=1) as pe, \
                         tc.tile_pool(name="psE", bufs=1, space="PSUM") as psE:
                        Gv = G_sum.rearrange("p (b c) -> p b c", b=HEADS)
                        nrm = pe.tile([48, 16], f32, tag="nrm", name="nrm")
                        inv = pe.tile([48, 16], f32, tag="inv", name="inv")
                        # cols 2b = qnorm, 2b+1 = knorm (payload-block order)
                        nc.scalar.sqrt(
                            nrm.rearrange("p (b u) -> p b u", b=HEADS),
                            Gv[:, :, 48:50])
                        nc.vector.tensor_scalar_max(nrm, nrm, 1e-12)
                        nc.vector.reciprocal(inv, nrm)

                        for h in range(HEADS):
                            b = blk(h)
                            B = pe.tile([48, 48], f32, tag="B", bufs=2,
                                        name=f"B{h}")
                            nc.vector.tensor_scalar(
                                out=B, in0=Gv[:, b, 0:48],
                                scalar1=inv[:, 2 * b + 1:2 * b + 2],
                                scalar2=float(temp[h]),
                                op0=MM, op1=MM)
                            psb = psE.tile([48, 48], f32, tag="ptrE", bufs=2,
                                           name=f"psb{h}")
                            nc.tensor.transpose(psb, B, id48_sb)
                            A0 = pe.tile([48, 48], f32, tag="A0", bufs=2,
                                         name=f"A0{h}")
                            nc.vector.tensor_scalar_mul(
                                A0, psb, inv[:, 2 * b:2 * b + 1])
                            e0 = pe.tile([48, 48], f32, tag="e0", bufs=2,
                                         name=f"e0{h}")
                            s_ = pe.tile([48, 4], f32, tag="s", bufs=2,
                                         name=f"s{h}")
                            nc.scalar.activation(e0, A0, ACT.Exp,
                                                 accum_out=s_[:, 0:1])
                            e1 = pe.tile([48, 48], f32, tag="e1", bufs=2,
                                         name=f"e1{h}")
                            nc.vector.tensor_mul(e1, e0,
                                                 mm_sb[:, 48 * h:48 * h + 48])
                            nc.vector.tensor_reduce(
                                s_[:, 1:2], e1, axis=AX, op=ADD)
                            r_ = pe.tile([48, 4], f32, tag="r", bufs=2,
                                         name=f"r{h}")
                            nc.vector.reciprocal(r_[:, 0:2], s_[:, 0:2])
                            nc.vector.tensor_scalar_mul(r_[:, 0:1], r_[:, 0:1],
                                                        float(wsm[0]))
                            nc.vector.tensor_scalar_mul(r_[:, 1:2], r_[:, 1:2],
                                                        float(wsm[1]))
                            t0 = pe.tile([48, 48], f32, tag="t0", bufs=2,
                                         name=f"t0{h}")
                            nc.vector.tensor_scalar_mul(t0, e0, r_[:, 0:1])
                            af = pe.tile([48, 48], f32, tag="af", bufs=2,
                                         name=f"af{h}")
                            nc.vector.tensor_scalar_mul(af, e1, r_[:, 1:2])
                            nc.vector.tensor_add(af, af, t0)
                            nc.vector.tensor_copy(attn_bf[h], af)

                        # M = proj @ blockdiag(attn), transposed into MT_sb
                        M_ps = [psE.tile([128, DIM], f32, tag=f"mps{ot}",
                                         name=f"mps{ot}") for ot in range(3)]
                        for h in range(HEADS):
                            for ot in range(3):
                                nc.tensor.matmul(
                                    M_ps[ot][:, 48 * h:48 * h + 48],
                                    lhsT=projT_sb[h][:, 128 * ot:128 * ot + 128],
                                    rhs=attn_bf[h], start=True, stop=True)
                        M_sb = []
                        for ot in range(3):
                            t = pe.tile([128, DIM], bf16, tag=f"msb{ot}",
                                        name=f"msb{ot}")
                            nc.vector.tensor_copy(t, M_ps[ot])
                            M_sb.append(t)
                        for ot in range(3):
                            for dtt in range(3):
                                psM = psE.tile([128, 128], bf16, tag="ptrE",
                                               bufs=2, name=f"psM{ot}_{dtt}")
                                nc.tensor.transpose(
                                    psM, M_sb[ot][:, 128 * dtt:128 * dtt + 128],
                                    ident_sb)
                                nc.vector.tensor_copy(
                                    MT_sb[dtt][:, 128 * ot:128 * ot + 128], psM)

                # ========== out chunks: fused pos-dw2 + M @ v ==========
                with tc.tile_pool(name="ph", bufs=1) as ph, \
                     tc.tile_pool(name="psH", bufs=1, space="PSUM") as psH:
                    vviews = [v_sb[i][:, PAD:PAD + NPOS].rearrange(
                        "p (r c) -> p r c", c=COLS) for i in range(3)]
                    g1views = [g1_sb[i][:, PAD:PAD + NPOS].rearrange(
                        "p (r c) -> p r c", c=COLS) for i in range(3)]
                    for ot in range(3):
                        for i in range(16):
                            psO = psH.tile([128, CH], f32, tag="out", bufs=2,
                                           name=f"psO{ot}_{i}")
                            for tap in range(9):
                                dy, dx = tap // 3 - 1, tap % 3 - 1
                                rhs = g1views[ot][:, 3 + 4 * i + dy:7 + 4 * i + dy,
                                                  1 + dx:129 + dx]
                                nc.tensor.matmul(
                                    psO, lhsT=dwd2_sb[ot][tap], rhs=rhs,
                                    start=(tap == 0), stop=False)
                            for dtt in range(3):
                                rhs = vviews[dtt][:, 3 + 4 * i:7 + 4 * i, 1:129]
                                nc.tensor.matmul(
                                    psO,
                                    lhsT=MT_sb[dtt][:, 128 * ot:128 * ot + 128],
                                    rhs=rhs, start=False, stop=(dtt == 2))
                            outf = ph.tile([128, CH], f32, tag="outf", bufs=3,
                                           name=f"outf{ot}_{i}")
                            nc.scalar.copy(outf, psO)
                            nc.sync.dma_start(
                                out_d.ap()[128 * ot:128 * ot + 128,
                                           CH * i:CH * i + CH], outf)

    nc.compile()
    return nc


def _host_prep(x, mask, qkv_w, dw_w, proj_w, temperature, w_blend,
               pos_w1, pos_w2):
    x = np.asarray(x, np.float32)
    b = x.shape[0]
    xp = np.zeros((b, DIM, 134, COLS), np.float32)
    xp[:, :, 3:131, 1:129] = x
    shards = []
    for core in range(NCORES):
        bi, s = core // 2, core % 2
        shards.append(np.ascontiguousarray(
            xp[bi, :, 64 * s:64 * s + ROWS, :]).reshape(DIM, NPOS).astype(BF))
    wT = np.ascontiguousarray(
        np.asarray(qkv_w, np.float32)[:, :, 0, 0].T).astype(BF)
    dwd = np.zeros((15, 9, 128, 128), np.float32)
    dwk = np.asarray(dw_w, np.float32)[:, 0]       # (1152, 3, 3)
    pk1 = np.asarray(pos_w1, np.float32)[:, 0]     # (384, 3, 3)
    pk2 = np.asarray(pos_w2, np.float32)[:, 0]
    idx = np.arange(128)
    for t9 in range(9):
        for tap in range(9):
            dwd[t9, tap, idx, idx] = dwk[128 * t9:128 * t9 + 128,
                                         tap // 3, tap % 3]
    for vt in range(3):
        for tap in range(9):
            dwd[9 + vt, tap, idx, idx] = pk1[128 * vt:128 * vt + 128,
                                             tap // 3, tap % 3]
            dwd[12 + vt, tap, idx, idx] = pk2[128 * vt:128 * vt + 128,
                                             tap // 3, tap % 3]
    dwd = dwd.astype(BF)
    pw = np.asarray(proj_w, np.float32)[:, :, 0, 0]
    projT = np.stack([np.ascontiguousarray(pw[:, 48 * h:48 * h + 48].T)
                      for h in range(HEADS)]).astype(BF)     # (8, 48, 384)
    mmul = (np.asarray(mask)[0] != 0).astype(np.float32)     # (8, 48, 48)
    maskmul = np.ascontiguousarray(
        mmul.transpose(1, 0, 2).reshape(48, HEADS * 48))
    ident = np.eye(128, dtype=np.float32).astype(BF)
    ident48 = np.eye(48, dtype=np.float32)
    wb = np.asarray(w_blend, np.float32)
    e = np.exp(wb - wb.max())
    wsm = e / e.sum()
    temp = np.asarray(temperature, np.float32).reshape(HEADS)
    return (shards, wT, dwd, projT, maskmul, ident, ident48, wsm, temp)


def kernel(**inputs):
    from concourse import bass_utils
    (shards, wT, dwd, projT, maskmul, ident, ident48, wsm,
     temp) = _host_prep(**inputs)
    key = (tuple(np.round(wsm, 8)), tuple(np.round(temp, 8)))
    if key not in _CACHE:
        _CACHE[key] = _build(wsm, temp)
    nc = _CACHE[key]
    in_maps = []
    for core in range(NCORES):
        s = core % 2
        ez = np.ones((128, 2), np.float32)
        ez[:, 0] = 0.0 if s == 0 else 1.0
        ez[:, 1] = 0.0 if s == 1 else 1.0
        in_maps.append({
            "x": shards[core], "wT": wT, "dwd": dwd,
            "projT": projT, "maskmul": maskmul, "ident": ident,
            "ident48": ident48, "ez": ez,
        })
    res = bass_utils.run_bass_kernel_spmd(
        nc, in_maps, core_ids=list(range(NCORES)),
        trace=bool(int(__import__("os").environ.get("KBENCH_TRACE", "0"))))
    kernel._last_result = res
    x = np.asarray(inputs["x"])
    out = np.zeros((x.shape[0], DIM, 128, 128), np.float32)
    for core in range(NCORES):
        bi, s = core // 2, core % 2
        out[bi, :, 64 * s:64 * s + 64, :] = \
            np.asarray(res.results[core]["out"], np.float32).reshape(
                DIM, 64, 128)
    return out
